# revision 12
# baseline (speedup 1.0000x reference)
"""Trainium2 (8 NeuronCores) kernel for:

    weight = sigmoid(x @ tokens^T)             # [N, T]
    masked = where(weight < 0.2, 0, weight)
    out    = x + masked @ tokens               # [N, D]

with x: [1000000, 128] f32, tokens: [64, 128] f32.

Strategy: pure data-parallel over the node dim N (125000 rows/core), tokens
replicated; no collectives needed in forward. The host pre-transposes each
x shard to xT [128, 125000] so the contraction dim d sits on SBUF partitions
— the whole device pipeline then runs in the transposed domain with zero
on-chip transposes and only the two tiny token matrices as matmul
stationaries:

    z_ps  [64, F] = tokensT.T @ xT_chunk      (f32, exact logits)
    s     [64, F] = sigmoid(z)                (ScalarE, PSUM -> SBUF)
    mw    [64, F] = (z >= logit(0.2)) * s     (one DVE scalar_tensor_tensor)
    yT_ps [128,F] = tokens.T @ mw             (f32)
    yT    [128,F] = yT_ps + xT_chunk          (DVE add, doubles as PSUM evac)

The host untransposes the gathered [128, 125000] outputs. Memory-bound:
~128 MB of HBM traffic per core, large contiguous per-partition DMAs.
"""

import numpy as np

import bass_rust
import concourse.bass as bass
import concourse.mybir as mybir
from concourse import tile
from concourse.bass_utils import run_bass_kernel_spmd
from concourse.tile import TileContext
from concourse.vector_clock import ScopedClock

# ---------------------------------------------------------------------------
# Workaround: this walrus build accepts only ONE sync-wait per instruction
# (setupSyncWait raises "Too many sync wait commands" otherwise), but Tile
# attaches one wait per depended-on proc. Legalize by inserting same-engine
# NoOp carriers, each holding one of the extra waits, in front of any
# multi-wait instruction before lowering; same trick for the kernel-tail
# drain, which is emitted after lowering.
# ---------------------------------------------------------------------------

_WSPLIT_COUNTER = [0]


def _split_waits_in_ordered(ordered):
    for bb_name, insts in ordered.items():
        new_list = []
        changed = False
        for inst in insts:
            si = getattr(inst, "sync_info", None)
            if si is not None and len(si.on_wait) > 1:
                waits = list(si.on_wait)
                for w in waits[:-1]:
                    nop = mybir.InstNoOp(
                        name=f"wsplit-{_WSPLIT_COUNTER[0]}", ins=[], outs=[]
                    )
                    _WSPLIT_COUNTER[0] += 1
                    nop.engine = inst.engine
                    nop.sync_info = bass_rust.SyncInfo(on_wait=[w], on_update=[])
                    new_list.append(nop)
                si.on_wait = [waits[-1]]
                inst.sync_info = si
                changed = True
            new_list.append(inst)
        if changed:
            ordered[bb_name] = new_list


_orig_lower_ordered = TileContext._lower_ordered_insts


def _patched_lower_ordered(self, ordered):
    _split_waits_in_ordered(ordered)
    return _orig_lower_ordered(self, ordered)


def _patched_drain_and_barrier(self, tick_clock, wait_clock):
    nc = self.nc
    drain_inst = nc.sync.drain()
    wait_clock.add_sem_waits(
        drain_inst.ins, ScopedClock({None: tick_clock.global_clock})
    )
    si = drain_inst.ins.sync_info
    if si is not None and len(si.on_wait) > 1:
        waits = list(si.on_wait)
        si.on_wait = [waits[0]]
        drain_inst.ins.sync_info = si
        for w in waits[1:]:
            carrier = nc.sync.drain()
            carrier.ins.sync_info = bass_rust.SyncInfo(on_wait=[w], on_update=[])

    nc.all_engine_barrier()
    assert self.sems is not None
    popped = nc._tile_sem_poison_stack.pop()
    assert popped is self._sem_poison
    nc.clear_and_free_semaphores(list(self.sems.allocated().values()))
    nc.all_engine_barrier()


TileContext._lower_ordered_insts = _patched_lower_ordered
TileContext._drain_and_barrier = _patched_drain_and_barrier

# ---------------------------------------------------------------------------
# Problem constants (hardcoded; the grading harness provides full inputs).
# ---------------------------------------------------------------------------

N_FULL, D, T = 1_000_000, 128, 64
N_CORES = 8
N_SHARD = N_FULL // N_CORES  # 125000
CHUNK = 2500                 # columns of xT per DMA (1.28 MiB in, 1.28 MiB out)
SUB = 500                    # columns per compute sub-chunk (one f32 matmul)
N_CHUNKS = N_SHARD // CHUNK  # 50, exact
THRESH_LOGIT = -1.3862943611198906  # logit(0.2); sigmoid(z) < 0.2 <=> z < this

# f32 matmuls cost 4x bf16 on the PE (2 cycles/col moving bandwidth x 2
# LOW/HIGH passes). The z logits keep full f32 so the threshold mask is
# exact; mm2 optionally runs bf16 (masked weights + tokens rounded to bf16).
MM1_BF16 = True
MM2_BF16 = True

F32 = mybir.dt.float32
BF16 = mybir.dt.bfloat16


def build_kernel() -> bass.Bass:
    nc = bass.Bass()
    mm1_dt = BF16 if MM1_BF16 else F32
    mm2_dt = BF16 if MM2_BF16 else F32
    xT_ext = nc.declare_dram_parameter("xT", [D, N_SHARD], F32, isOutput=False)
    tok_ext = nc.declare_dram_parameter("tokens", [T, D], mm2_dt, isOutput=False)
    tokT_ext = nc.declare_dram_parameter("tokensT", [D, T], mm1_dt, isOutput=False)
    out_ext = nc.declare_dram_parameter("out", [D, N_SHARD], F32, isOutput=True)

    with tile.TileContext(nc) as tc:
        with (
            tc.tile_pool(name="const", bufs=1) as cpool,
            tc.tile_pool(name="xin", bufs=3) as xpool,
            tc.tile_pool(name="yout", bufs=3) as ypool,
            tc.tile_pool(name="mask", bufs=4) as mpool,
            tc.tile_pool(name="ps_w", bufs=4, space="PSUM") as ps_w,
            tc.tile_pool(name="ps_y", bufs=4, space="PSUM") as ps_y,
        ):
            tok = cpool.tile([T, D], mm2_dt)
            nc.sync.dma_start(tok[:], tok_ext[:])
            tokT = cpool.tile([D, T], mm1_dt)
            nc.sync.dma_start(tokT[:], tokT_ext[:])

            for c in range(N_CHUNKS):
                lo = c * CHUNK
                xT = xpool.tile([D, CHUNK], F32, tag="xT")
                nc.sync.dma_start(xT[:], xT_ext[:, lo:lo + CHUNK])
                if MM1_BF16:
                    xT_bf = xpool.tile([D, CHUNK], BF16, tag="xT_bf")
                    nc.gpsimd.tensor_copy(xT_bf[:], xT[:])
                    mm1_rhs = xT_bf
                else:
                    mm1_rhs = xT
                yT = ypool.tile([D, CHUNK], F32, tag="yT")
                for s in range(CHUNK // SUB):
                    sl = slice(s * SUB, (s + 1) * SUB)
                    w_ps = ps_w.tile([T, SUB], F32)
                    nc.tensor.matmul(w_ps[:], tokT[:], mm1_rhs[:, sl])
                    s_sb = mpool.tile([T, SUB], F32, tag="s_sb")
                    nc.scalar.activation(
                        s_sb[:], w_ps[:], mybir.ActivationFunctionType.Sigmoid
                    )
                    mw = mpool.tile([T, SUB], mm2_dt, tag="mw")
                    nc.vector.scalar_tensor_tensor(
                        mw[:], w_ps[:], THRESH_LOGIT, s_sb[:],
                        mybir.AluOpType.is_ge, mybir.AluOpType.mult,
                    )
                    y_ps = ps_y.tile([D, SUB], F32)
                    nc.tensor.matmul(y_ps[:], tok[:], mw[:])
                    nc.vector.scalar_tensor_tensor(
                        yT[:, sl], y_ps[:], 0.0, xT[:, sl],
                        mybir.AluOpType.bypass, mybir.AluOpType.add,
                    )
                # scalar (qActDynamicHW) ring: overlap stores with sync-ring loads
                nc.scalar.dma_start(out_ext[:, lo:lo + CHUNK], yT[:])

    return nc


_NC_CACHE: bass.Bass | None = None


def _get_nc() -> bass.Bass:
    global _NC_CACHE
    if _NC_CACHE is None:
        _NC_CACHE = build_kernel()
    return _NC_CACHE


def run(x: np.ndarray, tokens: np.ndarray, **spmd_kwargs):
    import ml_dtypes

    nc = _get_nc()
    x = np.asarray(x, dtype=np.float32)
    tokens = np.ascontiguousarray(tokens, dtype=np.float32)
    tokensT = np.ascontiguousarray(tokens.T)
    if MM2_BF16:
        tokens = tokens.astype(ml_dtypes.bfloat16)
    if MM1_BF16:
        tokensT = tokensT.astype(ml_dtypes.bfloat16)
    in_maps = []
    for i in range(N_CORES):
        shard_T = np.ascontiguousarray(x[i * N_SHARD:(i + 1) * N_SHARD].T)
        in_maps.append({"xT": shard_T, "tokens": tokens, "tokensT": tokensT})
    res = run_bass_kernel_spmd(nc, in_maps, list(range(N_CORES)), **spmd_kwargs)
    out = np.empty((N_FULL, D), dtype=np.float32)
    for i in range(N_CORES):
        out[i * N_SHARD:(i + 1) * N_SHARD] = res.results[i]["out"].T
    return out, res


def kernel(x: np.ndarray, tokens: np.ndarray) -> np.ndarray:
    out, _ = run(x, tokens)
    return out


# revision 13
# speedup vs baseline: 1.2405x; 1.2405x over previous
"""Trainium2 (8 NeuronCores) kernel for:

    weight = sigmoid(x @ tokens^T)             # [N, T]
    masked = where(weight < 0.2, 0, weight)
    out    = x + masked @ tokens               # [N, D]

with x: [1000000, 128] f32, tokens: [64, 128] f32.

Strategy: pure data-parallel over the node dim N (125000 rows/core), tokens
replicated; no collectives needed in forward. The host pre-transposes each
x shard to xT [128, 125000] so the contraction dim d sits on SBUF partitions
— the whole device pipeline then runs in the transposed domain with zero
on-chip transposes and only the two tiny token matrices as matmul
stationaries:

    z_ps  [64, F] = tokensT.T @ xT_chunk      (f32, exact logits)
    s     [64, F] = sigmoid(z)                (ScalarE, PSUM -> SBUF)
    mw    [64, F] = (z >= logit(0.2)) * s     (one DVE scalar_tensor_tensor)
    yT_ps [128,F] = tokens.T @ mw             (f32)
    yT    [128,F] = yT_ps + xT_chunk          (DVE add, doubles as PSUM evac)

The host untransposes the gathered [128, 125000] outputs. Memory-bound:
~128 MB of HBM traffic per core, large contiguous per-partition DMAs.
"""

import numpy as np

import bass_rust
import concourse.bass as bass
import concourse.mybir as mybir
from concourse import tile
from concourse.bass_utils import run_bass_kernel_spmd
from concourse.tile import TileContext
from concourse.vector_clock import ScopedClock

# ---------------------------------------------------------------------------
# Workaround: this walrus build accepts only ONE sync-wait per instruction
# (setupSyncWait raises "Too many sync wait commands" otherwise), but Tile
# attaches one wait per depended-on proc. Legalize by inserting same-engine
# NoOp carriers, each holding one of the extra waits, in front of any
# multi-wait instruction before lowering; same trick for the kernel-tail
# drain, which is emitted after lowering.
# ---------------------------------------------------------------------------

_WSPLIT_COUNTER = [0]


def _split_waits_in_ordered(ordered):
    for bb_name, insts in ordered.items():
        new_list = []
        changed = False
        for inst in insts:
            si = getattr(inst, "sync_info", None)
            if si is not None and len(si.on_wait) > 1:
                waits = list(si.on_wait)
                for w in waits[:-1]:
                    nop = mybir.InstNoOp(
                        name=f"wsplit-{_WSPLIT_COUNTER[0]}", ins=[], outs=[]
                    )
                    _WSPLIT_COUNTER[0] += 1
                    nop.engine = inst.engine
                    nop.sync_info = bass_rust.SyncInfo(on_wait=[w], on_update=[])
                    new_list.append(nop)
                si.on_wait = [waits[-1]]
                inst.sync_info = si
                changed = True
            new_list.append(inst)
        if changed:
            ordered[bb_name] = new_list


_orig_lower_ordered = TileContext._lower_ordered_insts


def _patched_lower_ordered(self, ordered):
    _split_waits_in_ordered(ordered)
    return _orig_lower_ordered(self, ordered)


def _patched_drain_and_barrier(self, tick_clock, wait_clock):
    nc = self.nc
    drain_inst = nc.sync.drain()
    wait_clock.add_sem_waits(
        drain_inst.ins, ScopedClock({None: tick_clock.global_clock})
    )
    si = drain_inst.ins.sync_info
    if si is not None and len(si.on_wait) > 1:
        waits = list(si.on_wait)
        si.on_wait = [waits[0]]
        drain_inst.ins.sync_info = si
        for w in waits[1:]:
            carrier = nc.sync.drain()
            carrier.ins.sync_info = bass_rust.SyncInfo(on_wait=[w], on_update=[])

    nc.all_engine_barrier()
    assert self.sems is not None
    popped = nc._tile_sem_poison_stack.pop()
    assert popped is self._sem_poison
    nc.clear_and_free_semaphores(list(self.sems.allocated().values()))
    nc.all_engine_barrier()


TileContext._lower_ordered_insts = _patched_lower_ordered
TileContext._drain_and_barrier = _patched_drain_and_barrier

# ---------------------------------------------------------------------------
# Problem constants (hardcoded; the grading harness provides full inputs).
# ---------------------------------------------------------------------------

N_FULL, D, T = 1_000_000, 128, 64
N_CORES = 8
N_SHARD = N_FULL // N_CORES  # 125000
CHUNK = 2500                 # columns of xT per DMA (1.28 MiB in, 1.28 MiB out)
SUB = 500                    # columns per compute sub-chunk (one f32 matmul)
N_CHUNKS = N_SHARD // CHUNK  # 50, exact
THRESH_LOGIT = -1.3862943611198906  # logit(0.2); sigmoid(z) < 0.2 <=> z < this

# f32 matmuls cost 4x bf16 on the PE (2 cycles/col moving bandwidth x 2
# LOW/HIGH passes). The z logits keep full f32 so the threshold mask is
# exact; mm2 optionally runs bf16 (masked weights + tokens rounded to bf16).
MM1_BF16 = False
MM2_BF16 = True

F32 = mybir.dt.float32
BF16 = mybir.dt.bfloat16


def build_kernel() -> bass.Bass:
    nc = bass.Bass()
    mm1_dt = BF16 if MM1_BF16 else F32
    mm2_dt = BF16 if MM2_BF16 else F32
    xT_ext = nc.declare_dram_parameter("xT", [D, N_SHARD], F32, isOutput=False)
    tok_ext = nc.declare_dram_parameter("tokens", [T, D], mm2_dt, isOutput=False)
    tokT_ext = nc.declare_dram_parameter("tokensT", [D, T], mm1_dt, isOutput=False)
    out_ext = nc.declare_dram_parameter("out", [D, N_SHARD], F32, isOutput=True)

    with tile.TileContext(nc) as tc:
        with (
            tc.tile_pool(name="const", bufs=1) as cpool,
            tc.tile_pool(name="xin", bufs=3) as xpool,
            tc.tile_pool(name="yout", bufs=3) as ypool,
            tc.tile_pool(name="mask", bufs=4) as mpool,
            tc.tile_pool(name="ps_w", bufs=4, space="PSUM") as ps_w,
            tc.tile_pool(name="ps_y", bufs=4, space="PSUM") as ps_y,
        ):
            tok = cpool.tile([T, D], mm2_dt)
            nc.sync.dma_start(tok[:], tok_ext[:])
            tokT = cpool.tile([D, T], mm1_dt)
            nc.sync.dma_start(tokT[:], tokT_ext[:])

            for c in range(N_CHUNKS):
                lo = c * CHUNK
                xT = xpool.tile([D, CHUNK], F32, tag="xT")
                nc.sync.dma_start(xT[:], xT_ext[:, lo:lo + CHUNK])
                if MM1_BF16:
                    xT_bf = xpool.tile([D, CHUNK], BF16, tag="xT_bf")
                    nc.gpsimd.tensor_copy(xT_bf[:], xT[:])
                    mm1_rhs = xT_bf
                else:
                    mm1_rhs = xT
                yT = ypool.tile([D, CHUNK], F32, tag="yT")
                for s in range(CHUNK // SUB):
                    sl = slice(s * SUB, (s + 1) * SUB)
                    w_ps = ps_w.tile([T, SUB], F32)
                    nc.tensor.matmul(w_ps[:], tokT[:], mm1_rhs[:, sl])
                    s_sb = mpool.tile([T, SUB], F32, tag="s_sb")
                    nc.scalar.activation(
                        s_sb[:], w_ps[:], mybir.ActivationFunctionType.Sigmoid
                    )
                    mw = mpool.tile([T, SUB], mm2_dt, tag="mw")
                    nc.vector.scalar_tensor_tensor(
                        mw[:], w_ps[:], THRESH_LOGIT, s_sb[:],
                        mybir.AluOpType.is_ge, mybir.AluOpType.mult,
                    )
                    y_ps = ps_y.tile([D, SUB], F32)
                    nc.tensor.matmul(y_ps[:], tok[:], mw[:])
                    nc.vector.scalar_tensor_tensor(
                        yT[:, sl], y_ps[:], 0.0, xT[:, sl],
                        mybir.AluOpType.bypass, mybir.AluOpType.add,
                    )
                # scalar (qActDynamicHW) ring: overlap stores with sync-ring
                # loads; two half-chunk stores drain results earlier
                h = CHUNK // 2
                nc.scalar.dma_start(out_ext[:, lo:lo + h], yT[:, :h])
                nc.scalar.dma_start(out_ext[:, lo + h:lo + CHUNK], yT[:, h:])

    return nc


_NC_CACHE: bass.Bass | None = None


def _get_nc() -> bass.Bass:
    global _NC_CACHE
    if _NC_CACHE is None:
        _NC_CACHE = build_kernel()
    return _NC_CACHE


def run(x: np.ndarray, tokens: np.ndarray, **spmd_kwargs):
    import ml_dtypes

    nc = _get_nc()
    x = np.asarray(x, dtype=np.float32)
    tokens = np.ascontiguousarray(tokens, dtype=np.float32)
    tokensT = np.ascontiguousarray(tokens.T)
    if MM2_BF16:
        tokens = tokens.astype(ml_dtypes.bfloat16)
    if MM1_BF16:
        tokensT = tokensT.astype(ml_dtypes.bfloat16)
    in_maps = []
    for i in range(N_CORES):
        shard_T = np.ascontiguousarray(x[i * N_SHARD:(i + 1) * N_SHARD].T)
        in_maps.append({"xT": shard_T, "tokens": tokens, "tokensT": tokensT})
    res = run_bass_kernel_spmd(nc, in_maps, list(range(N_CORES)), **spmd_kwargs)
    out = np.empty((N_FULL, D), dtype=np.float32)
    for i in range(N_CORES):
        out[i * N_SHARD:(i + 1) * N_SHARD] = res.results[i]["out"].T
    return out, res


def kernel(x: np.ndarray, tokens: np.ndarray) -> np.ndarray:
    out, _ = run(x, tokens)
    return out


# revision 14
# speedup vs baseline: 1.4050x; 1.1326x over previous
"""Trainium2 (8 NeuronCores) kernel for:

    weight = sigmoid(x @ tokens^T)             # [N, T]
    masked = where(weight < 0.2, 0, weight)
    out    = x + masked @ tokens               # [N, D]

with x: [1000000, 128] f32, tokens: [64, 128] f32.

Strategy: pure data-parallel over the node dim N (125000 rows/core), tokens
replicated; no collectives needed in forward. The host pre-transposes each
x shard to xT [128, 125000] so the contraction dim d sits on SBUF partitions
— the whole device pipeline then runs in the transposed domain with zero
on-chip transposes and only the two tiny token matrices as matmul
stationaries:

    z_ps  [64, F] = tokensT.T @ xT_chunk      (f32, exact logits)
    s     [64, F] = sigmoid(z)                (ScalarE, PSUM -> SBUF)
    mw    [64, F] = (z >= logit(0.2)) * s     (one DVE scalar_tensor_tensor)
    yT_ps [128,F] = tokens.T @ mw             (f32)
    yT    [128,F] = yT_ps + xT_chunk          (DVE add, doubles as PSUM evac)

The host untransposes the gathered [128, 125000] outputs. Memory-bound:
~128 MB of HBM traffic per core, large contiguous per-partition DMAs.
"""

import numpy as np

import bass_rust
import concourse.bass as bass
import concourse.mybir as mybir
from concourse import tile
from concourse.bass_utils import run_bass_kernel_spmd
from concourse.tile import TileContext
from concourse.vector_clock import ScopedClock

# ---------------------------------------------------------------------------
# Workaround: this walrus build accepts only ONE sync-wait per instruction
# (setupSyncWait raises "Too many sync wait commands" otherwise), but Tile
# attaches one wait per depended-on proc. Legalize by inserting same-engine
# NoOp carriers, each holding one of the extra waits, in front of any
# multi-wait instruction before lowering; same trick for the kernel-tail
# drain, which is emitted after lowering.
# ---------------------------------------------------------------------------

_WSPLIT_COUNTER = [0]


def _split_waits_in_ordered(ordered):
    for bb_name, insts in ordered.items():
        new_list = []
        changed = False
        for inst in insts:
            si = getattr(inst, "sync_info", None)
            if si is not None and len(si.on_wait) > 1:
                waits = list(si.on_wait)
                for w in waits[:-1]:
                    nop = mybir.InstNoOp(
                        name=f"wsplit-{_WSPLIT_COUNTER[0]}", ins=[], outs=[]
                    )
                    _WSPLIT_COUNTER[0] += 1
                    nop.engine = inst.engine
                    nop.sync_info = bass_rust.SyncInfo(on_wait=[w], on_update=[])
                    new_list.append(nop)
                si.on_wait = [waits[-1]]
                inst.sync_info = si
                changed = True
            new_list.append(inst)
        if changed:
            ordered[bb_name] = new_list


_orig_lower_ordered = TileContext._lower_ordered_insts


def _patched_lower_ordered(self, ordered):
    _split_waits_in_ordered(ordered)
    return _orig_lower_ordered(self, ordered)


def _patched_drain_and_barrier(self, tick_clock, wait_clock):
    nc = self.nc
    drain_inst = nc.sync.drain()
    wait_clock.add_sem_waits(
        drain_inst.ins, ScopedClock({None: tick_clock.global_clock})
    )
    si = drain_inst.ins.sync_info
    if si is not None and len(si.on_wait) > 1:
        waits = list(si.on_wait)
        si.on_wait = [waits[0]]
        drain_inst.ins.sync_info = si
        for w in waits[1:]:
            carrier = nc.sync.drain()
            carrier.ins.sync_info = bass_rust.SyncInfo(on_wait=[w], on_update=[])

    nc.all_engine_barrier()
    assert self.sems is not None
    popped = nc._tile_sem_poison_stack.pop()
    assert popped is self._sem_poison
    nc.clear_and_free_semaphores(list(self.sems.allocated().values()))
    nc.all_engine_barrier()


TileContext._lower_ordered_insts = _patched_lower_ordered
TileContext._drain_and_barrier = _patched_drain_and_barrier

# ---------------------------------------------------------------------------
# Problem constants (hardcoded; the grading harness provides full inputs).
# ---------------------------------------------------------------------------

N_FULL, D, T = 1_000_000, 128, 64
N_CORES = 8
N_SHARD = N_FULL // N_CORES  # 125000
CHUNK = 5000                 # columns of xT per DMA (2.56 MiB in)
SUB = 500                    # columns per compute sub-chunk (one f32 matmul)
N_CHUNKS = N_SHARD // CHUNK  # 25, exact
THRESH_LOGIT = -1.3862943611198906  # logit(0.2); sigmoid(z) < 0.2 <=> z < this

# f32 matmuls cost 4x bf16 on the PE (2 cycles/col moving bandwidth x 2
# LOW/HIGH passes). The z logits keep full f32 so the threshold mask is
# exact; mm2 optionally runs bf16 (masked weights + tokens rounded to bf16).
MM1_BF16 = False
MM2_BF16 = True

F32 = mybir.dt.float32
BF16 = mybir.dt.bfloat16


def build_kernel() -> bass.Bass:
    nc = bass.Bass()
    mm1_dt = BF16 if MM1_BF16 else F32
    mm2_dt = BF16 if MM2_BF16 else F32
    xT_ext = nc.declare_dram_parameter("xT", [D, N_SHARD], F32, isOutput=False)
    tok_ext = nc.declare_dram_parameter("tokens", [T, D], mm2_dt, isOutput=False)
    tokT_ext = nc.declare_dram_parameter("tokensT", [D, T], mm1_dt, isOutput=False)
    out_ext = nc.declare_dram_parameter("out", [D, N_SHARD], F32, isOutput=True)

    with tile.TileContext(nc) as tc:
        with (
            tc.tile_pool(name="const", bufs=1) as cpool,
            tc.tile_pool(name="xin", bufs=3) as xpool,
            tc.tile_pool(name="yout", bufs=3) as ypool,
            tc.tile_pool(name="mask", bufs=4) as mpool,
            tc.tile_pool(name="ps_w", bufs=4, space="PSUM") as ps_w,
            tc.tile_pool(name="ps_y", bufs=4, space="PSUM") as ps_y,
        ):
            tok = cpool.tile([T, D], mm2_dt)
            nc.sync.dma_start(tok[:], tok_ext[:])
            tokT = cpool.tile([D, T], mm1_dt)
            nc.sync.dma_start(tokT[:], tokT_ext[:])

            for c in range(N_CHUNKS):
                lo = c * CHUNK
                xT = xpool.tile([D, CHUNK], F32, tag="xT")
                nc.sync.dma_start(xT[:], xT_ext[:, lo:lo + CHUNK])
                if MM1_BF16:
                    xT_bf = xpool.tile([D, CHUNK], BF16, tag="xT_bf")
                    nc.gpsimd.tensor_copy(xT_bf[:], xT[:])
                    mm1_rhs = xT_bf
                else:
                    mm1_rhs = xT
                yT = ypool.tile([D, CHUNK], F32, tag="yT")
                for s in range(CHUNK // SUB):
                    sl = slice(s * SUB, (s + 1) * SUB)
                    w_ps = ps_w.tile([T, SUB], F32)
                    nc.tensor.matmul(w_ps[:], tokT[:], mm1_rhs[:, sl])
                    s_sb = mpool.tile([T, SUB], F32, tag="s_sb")
                    nc.scalar.activation(
                        s_sb[:], w_ps[:], mybir.ActivationFunctionType.Sigmoid
                    )
                    mw = mpool.tile([T, SUB], mm2_dt, tag="mw")
                    nc.vector.scalar_tensor_tensor(
                        mw[:], w_ps[:], THRESH_LOGIT, s_sb[:],
                        mybir.AluOpType.is_ge, mybir.AluOpType.mult,
                    )
                    y_ps = ps_y.tile([D, SUB], F32)
                    nc.tensor.matmul(y_ps[:], tok[:], mw[:])
                    nc.vector.scalar_tensor_tensor(
                        yT[:, sl], y_ps[:], 0.0, xT[:, sl],
                        mybir.AluOpType.bypass, mybir.AluOpType.add,
                    )
                # scalar (qActDynamicHW) ring: overlap stores with sync-ring
                # loads; quarter-chunk stores drain results earlier
                q = CHUNK // 4
                for k in range(4):
                    nc.scalar.dma_start(
                        out_ext[:, lo + k * q:lo + (k + 1) * q],
                        yT[:, k * q:(k + 1) * q],
                    )

    return nc


_NC_CACHE: bass.Bass | None = None


def _get_nc() -> bass.Bass:
    global _NC_CACHE
    if _NC_CACHE is None:
        _NC_CACHE = build_kernel()
    return _NC_CACHE


def run(x: np.ndarray, tokens: np.ndarray, **spmd_kwargs):
    import ml_dtypes

    nc = _get_nc()
    x = np.asarray(x, dtype=np.float32)
    tokens = np.ascontiguousarray(tokens, dtype=np.float32)
    tokensT = np.ascontiguousarray(tokens.T)
    if MM2_BF16:
        tokens = tokens.astype(ml_dtypes.bfloat16)
    if MM1_BF16:
        tokensT = tokensT.astype(ml_dtypes.bfloat16)
    in_maps = []
    for i in range(N_CORES):
        shard_T = np.ascontiguousarray(x[i * N_SHARD:(i + 1) * N_SHARD].T)
        in_maps.append({"xT": shard_T, "tokens": tokens, "tokensT": tokensT})
    res = run_bass_kernel_spmd(nc, in_maps, list(range(N_CORES)), **spmd_kwargs)
    out = np.empty((N_FULL, D), dtype=np.float32)
    for i in range(N_CORES):
        out[i * N_SHARD:(i + 1) * N_SHARD] = res.results[i]["out"].T
    return out, res


def kernel(x: np.ndarray, tokens: np.ndarray) -> np.ndarray:
    out, _ = run(x, tokens)
    return out


# revision 18
# speedup vs baseline: 1.4767x; 1.0510x over previous
"""Trainium2 (8 NeuronCores) kernel for:

    weight = sigmoid(x @ tokens^T)             # [N, T]
    masked = where(weight < 0.2, 0, weight)
    out    = x + masked @ tokens               # [N, D]

with x: [1000000, 128] f32, tokens: [64, 128] f32.

Strategy: pure data-parallel over the node dim N (125000 rows/core), tokens
replicated; no collectives needed in forward. The host pre-transposes each
x shard to xT [128, 125000] so the contraction dim d sits on SBUF partitions
— the whole device pipeline then runs in the transposed domain with zero
on-chip transposes and only the two tiny token matrices as matmul
stationaries:

    z_ps  [64, F] = tokensT.T @ xT_chunk      (f32, exact logits)
    s     [64, F] = sigmoid(z)                (ScalarE, PSUM -> SBUF)
    mw    [64, F] = (z >= logit(0.2)) * s     (one DVE scalar_tensor_tensor)
    yT_ps [128,F] = tokens.T @ mw             (f32)
    yT    [128,F] = yT_ps + xT_chunk          (DVE add, doubles as PSUM evac)

The host untransposes the gathered [128, 125000] outputs. Memory-bound:
~128 MB of HBM traffic per core, large contiguous per-partition DMAs.
"""

import numpy as np

import bass_rust
import concourse.bass as bass
import concourse.mybir as mybir
from concourse import tile
from concourse.bass_utils import run_bass_kernel_spmd
from concourse.tile import TileContext
from concourse.vector_clock import ScopedClock

# ---------------------------------------------------------------------------
# Workaround: this walrus build accepts only ONE sync-wait per instruction
# (setupSyncWait raises "Too many sync wait commands" otherwise), but Tile
# attaches one wait per depended-on proc. Legalize by inserting same-engine
# NoOp carriers, each holding one of the extra waits, in front of any
# multi-wait instruction before lowering; same trick for the kernel-tail
# drain, which is emitted after lowering.
# ---------------------------------------------------------------------------

_WSPLIT_COUNTER = [0]


def _split_waits_in_ordered(ordered):
    for bb_name, insts in ordered.items():
        new_list = []
        changed = False
        for inst in insts:
            si = getattr(inst, "sync_info", None)
            if si is not None and len(si.on_wait) > 1:
                waits = list(si.on_wait)
                for w in waits[:-1]:
                    nop = mybir.InstNoOp(
                        name=f"wsplit-{_WSPLIT_COUNTER[0]}", ins=[], outs=[]
                    )
                    _WSPLIT_COUNTER[0] += 1
                    nop.engine = inst.engine
                    nop.sync_info = bass_rust.SyncInfo(on_wait=[w], on_update=[])
                    new_list.append(nop)
                si.on_wait = [waits[-1]]
                inst.sync_info = si
                changed = True
            new_list.append(inst)
        if changed:
            ordered[bb_name] = new_list


_orig_lower_ordered = TileContext._lower_ordered_insts


def _patched_lower_ordered(self, ordered):
    _split_waits_in_ordered(ordered)
    return _orig_lower_ordered(self, ordered)


def _patched_drain_and_barrier(self, tick_clock, wait_clock):
    nc = self.nc
    drain_inst = nc.sync.drain()
    wait_clock.add_sem_waits(
        drain_inst.ins, ScopedClock({None: tick_clock.global_clock})
    )
    si = drain_inst.ins.sync_info
    if si is not None and len(si.on_wait) > 1:
        waits = list(si.on_wait)
        si.on_wait = [waits[0]]
        drain_inst.ins.sync_info = si
        for w in waits[1:]:
            carrier = nc.sync.drain()
            carrier.ins.sync_info = bass_rust.SyncInfo(on_wait=[w], on_update=[])

    nc.all_engine_barrier()
    assert self.sems is not None
    popped = nc._tile_sem_poison_stack.pop()
    assert popped is self._sem_poison
    nc.clear_and_free_semaphores(list(self.sems.allocated().values()))
    nc.all_engine_barrier()


TileContext._lower_ordered_insts = _patched_lower_ordered
TileContext._drain_and_barrier = _patched_drain_and_barrier

# ---------------------------------------------------------------------------
# Problem constants (hardcoded; the grading harness provides full inputs).
# ---------------------------------------------------------------------------

N_FULL, D, T = 1_000_000, 128, 64
N_CORES = 8
N_SHARD = N_FULL // N_CORES  # 125000
CHUNK = 5000                 # columns of xT per DMA (2.56 MiB in)
SUB = 500                    # columns per compute sub-chunk (one f32 matmul)
N_CHUNKS = N_SHARD // CHUNK  # 25, exact
THRESH_LOGIT = -1.3862943611198906  # logit(0.2); sigmoid(z) < 0.2 <=> z < this

# f32 matmuls cost 4x bf16 on the PE (2 cycles/col moving bandwidth x 2
# LOW/HIGH passes). The z logits keep full f32 so the threshold mask is
# exact; mm2 optionally runs bf16 (masked weights + tokens rounded to bf16).
MM1_BF16 = False
MM2_BF16 = True

F32 = mybir.dt.float32
BF16 = mybir.dt.bfloat16


def build_kernel() -> bass.Bass:
    nc = bass.Bass()
    mm1_dt = BF16 if MM1_BF16 else F32
    mm2_dt = BF16 if MM2_BF16 else F32
    xT_ext = nc.declare_dram_parameter("xT", [D, N_SHARD], F32, isOutput=False)
    tok_ext = nc.declare_dram_parameter("tokens", [T, D], mm2_dt, isOutput=False)
    tokT_ext = nc.declare_dram_parameter("tokensT", [D, T], mm1_dt, isOutput=False)
    out_ext = nc.declare_dram_parameter("out", [D, N_SHARD], F32, isOutput=True)

    with tile.TileContext(nc) as tc:
        with (
            tc.tile_pool(name="const", bufs=1) as cpool,
            tc.tile_pool(name="xin", bufs=3) as xpool,
            tc.tile_pool(name="yout", bufs=3) as ypool,
            tc.tile_pool(name="mask", bufs=4) as mpool,
            tc.tile_pool(name="ps_w", bufs=4, space="PSUM") as ps_w,
            tc.tile_pool(name="ps_y", bufs=4, space="PSUM") as ps_y,
        ):
            tok = cpool.tile([T, D], mm2_dt)
            nc.sync.dma_start(tok[:], tok_ext[:])
            tokT = cpool.tile([D, T], mm1_dt)
            nc.sync.dma_start(tokT[:], tokT_ext[:])

            for c in range(N_CHUNKS):
                lo = c * CHUNK
                xT = xpool.tile([D, CHUNK], F32, tag="xT")
                if c == 0:
                    # quarter-loads let compute start ~3x earlier on the
                    # first chunk (ramp-up)
                    q0 = CHUNK // 4
                    for k in range(4):
                        nc.sync.dma_start(
                            xT[:, k * q0:(k + 1) * q0],
                            xT_ext[:, lo + k * q0:lo + (k + 1) * q0],
                        )
                else:
                    nc.sync.dma_start(xT[:], xT_ext[:, lo:lo + CHUNK])
                if MM1_BF16:
                    xT_bf = xpool.tile([D, CHUNK], BF16, tag="xT_bf")
                    nc.gpsimd.tensor_copy(xT_bf[:], xT[:])
                    mm1_rhs = xT_bf
                else:
                    mm1_rhs = xT
                yT = ypool.tile([D, CHUNK], F32, tag="yT")
                for s in range(CHUNK // SUB):
                    sl = slice(s * SUB, (s + 1) * SUB)
                    w_ps = ps_w.tile([T, SUB], F32)
                    nc.tensor.matmul(w_ps[:], tokT[:], mm1_rhs[:, sl])
                    s_sb = mpool.tile([T, SUB], F32, tag="s_sb")
                    nc.scalar.activation(
                        s_sb[:], w_ps[:], mybir.ActivationFunctionType.Sigmoid
                    )
                    mw = mpool.tile([T, SUB], mm2_dt, tag="mw")
                    nc.vector.scalar_tensor_tensor(
                        mw[:], w_ps[:], THRESH_LOGIT, s_sb[:],
                        mybir.AluOpType.is_ge, mybir.AluOpType.mult,
                    )
                    y_ps = ps_y.tile([D, SUB], F32)
                    nc.tensor.matmul(y_ps[:], tok[:], mw[:])
                    nc.vector.scalar_tensor_tensor(
                        yT[:, sl], y_ps[:], 0.0, xT[:, sl],
                        mybir.AluOpType.bypass, mybir.AluOpType.add,
                    )
                # scalar (qActDynamicHW) ring: overlap stores with sync-ring
                # loads; quarter-chunk stores drain results earlier
                q = CHUNK // 4
                for k in range(4):
                    nc.scalar.dma_start(
                        out_ext[:, lo + k * q:lo + (k + 1) * q],
                        yT[:, k * q:(k + 1) * q],
                    )

    return nc


_NC_CACHE: bass.Bass | None = None


def _get_nc() -> bass.Bass:
    global _NC_CACHE
    if _NC_CACHE is None:
        _NC_CACHE = build_kernel()
    return _NC_CACHE


def run(x: np.ndarray, tokens: np.ndarray, **spmd_kwargs):
    import ml_dtypes

    nc = _get_nc()
    x = np.asarray(x, dtype=np.float32)
    tokens = np.ascontiguousarray(tokens, dtype=np.float32)
    tokensT = np.ascontiguousarray(tokens.T)
    if MM2_BF16:
        tokens = tokens.astype(ml_dtypes.bfloat16)
    if MM1_BF16:
        tokensT = tokensT.astype(ml_dtypes.bfloat16)
    in_maps = []
    for i in range(N_CORES):
        shard_T = np.ascontiguousarray(x[i * N_SHARD:(i + 1) * N_SHARD].T)
        in_maps.append({"xT": shard_T, "tokens": tokens, "tokensT": tokensT})
    res = run_bass_kernel_spmd(nc, in_maps, list(range(N_CORES)), **spmd_kwargs)
    out = np.empty((N_FULL, D), dtype=np.float32)
    for i in range(N_CORES):
        out[i * N_SHARD:(i + 1) * N_SHARD] = res.results[i]["out"].T
    return out, res


def kernel(x: np.ndarray, tokens: np.ndarray) -> np.ndarray:
    out, _ = run(x, tokens)
    return out
